# revision 2
# baseline (speedup 1.0000x reference)
"""Trainium2 Bass kernel for relative-position multi-head attention.

Shapes (hardcoded): B=2, L=384, D=256, H=8, DH=32.
Sharding: 8 cores; core c handles batch b=c//4, query rows [(c%4)*96, +96).
Pure data-parallel SPMD - no collectives.

Math (per batch b, query q):
  q/k/v projections: x @ W.T + bias
  A_C[h,k] = (q_h+u_h) . k_h[k]
  B_D[h,k] = (q_h+v_h) . (Wr_h @ pos[q,k] + br_h)
           = (Wr_h^T (q_h+v_h)) . pos[q,k]   + const(h,q)   [br term is
             k-independent -> cancels in softmax -> dropped]
  score    = (A_C + B_D)/sqrt(DH) - (1-mask[k])*1e15
  out      = softmax_k(score) @ v

Key restructurings:
  * r = pos @ Wr.T (38 GFLOP) is never materialized; instead
    T[q] = Wr^T-blockdiag @ (q+v)  (a [256,8] matrix per query) and
    B_D = posT @ T  (1.2 GFLOP).
  * ALL layout work happens on the host in shard_inputs: pos arrives as
    bf16 [128(d%128), Q, 2(d//128), L] (partition-major, so each pos DMA
    is a clean per-partition-contiguous stream), and every weight arrives
    pre-transposed/packed. The device runs only: projections, the 576
    B_D matmuls (6 per query pair: 2 d-blocks x 3 k-tiles, each
    LDW[128x128 bf16] + MM[N=8]), fused exp, and the output contraction.
  * scores live in PSUM as [k-partitions, (pair,h)-free]; softmax over k
    (partitions) uses exp on ACT + a ones-column appended to v_proj so the
    softmax denominator falls out of the output matmul for free. A_C is
    folded into the same PSUM accumulation (opens the group), B_D closes.
"""

import sys

for _p in ("/opt/trn_rl_repo", "/root/.axon_site/_ro/trn_rl_repo"):
    if _p not in sys.path:
        sys.path.append(_p)

import numpy as np

import concourse.bass as bass
import concourse.mybir as mybir
import concourse.tile as tile
from concourse import bacc
from concourse.masks import make_identity

FP32 = mybir.dt.float32
BF16 = mybir.dt.bfloat16

B, L, D, H = 2, 384, 256, 8
DH = D // H            # 32
Q = 96                 # queries per core
KT = L // 128          # 3 k-tiles
CB = D // 128          # 2 contraction blocks
NCORES = 8
SCALE = 1.0 / np.sqrt(DH)
PG = 8                 # pairs per pos DMA group
NG = Q // PG           # 12 groups

# column layout of the host-packed weight tensor "wb" [D, WBC]
WB_WK, WB_WQ, WB_WV = 0, D, 2 * D      # Wk.T / Wq.T / Wv.T   [D, D]
WB_KEY, WB_VAL = 3 * D, 3 * D + L      # key.T / value.T      [D, L]
WB_QRY = 3 * D + 2 * L                 # query.T              [D, Q]
WBC = 3 * D + 2 * L + Q                # 1632

# "cols" [128, 12] f32 per-partition columns:
#   0,1 = bk | 2,3 = bq+u | 4,5 = bq+v | 6,7,8 = (mask-1)*1e15 per k-tile


def build_kernel_body(tc, outs, ins):
    """Emit the per-core program. outs/ins are dicts of DRAM APs."""
    from contextlib import ExitStack
    ctx = ExitStack()
    pool = lambda **kw: ctx.enter_context(tc.tile_pool(**kw))
    nc = tc.nc
    posT = ins["posT"]      # [128, Q*CB*L] bf16  (p, q, cb, k)
    wb = ins["wb"]          # [D, WBC] f32
    wrh = ins["wrh"]        # [DH, H*D] f32 : wrh[i, h*D+d] = Wr[h*DH+i, d]
    cols = ins["cols"]      # [128, 12] f32
    bvrow = ins["bvrow"]    # [1, D] f32
    out = outs["out"]       # [Q, D] f32

    const = pool(name="const", bufs=1)
    work = pool(name="work", bufs=2)
    pos_pool = pool(name="pos", bufs=4)
    psum_big = pool(name="psum_big", bufs=3, space="PSUM")
    psum_out = pool(name="psum_out", bufs=1, space="PSUM")

    # ---------------- setup loads (Act ring; pos stream owns sync) ------
    wb_t = [const.tile([128, WBC], FP32, tag=f"wb{cb}", name=f"wb{cb}")
            for cb in range(CB)]
    for cb in range(CB):
        nc.scalar.dma_start(out=wb_t[cb], in_=wb[cb * 128:(cb + 1) * 128, :])
    wrh_t = const.tile([DH, H * D], FP32, tag="wrh", name="wrh")
    nc.scalar.dma_start(out=wrh_t, in_=wrh)
    cols_t = const.tile([128, 12], FP32, tag="cols", name="cols")
    nc.scalar.dma_start(out=cols_t, in_=cols)
    bv_row = const.tile([1, D], FP32, tag="bvrow", name="bvrow")
    nc.scalar.dma_start(out=bv_row, in_=bvrow)

    ident_f = const.tile([128, 128], FP32, tag="idf", name="idf")
    make_identity(nc, ident_f)

    def wslice(cb, base, j):  # [128, 128] lhsT slice of the packed weights
        return wb_t[cb][:, base + j * 128: base + (j + 1) * 128]

    # ---------------- projections ----------------
    # k_projT per-head [32, L] bf16 (+bk); matmul lhsT base must be 0/32/64
    kp_h = [const.tile([DH, L], BF16, tag=f"kph{h}", name=f"kph{h}")
            for h in range(H)]
    for dt in range(CB):
        ps = psum_big.tile([128, 1024], FP32, tag="big", name="ps_kp")
        for cb in range(CB):
            nc.tensor.matmul(
                ps[:, :L], wslice(cb, WB_WK, dt),
                wb_t[cb][:, WB_KEY:WB_KEY + L],
                start=(cb == 0), stop=(cb == CB - 1))
        for hh in range(4):
            h = dt * 4 + hh
            nc.vector.tensor_scalar_add(
                out=kp_h[h], in0=ps[hh * DH:(hh + 1) * DH, :L],
                scalar1=cols_t[hh * DH:(hh + 1) * DH, 0 + dt:1 + dt])

    # q_projT [d', q]; qu = +bq+u, qv = +bq+v (per-partition adds)
    qu_hb = [const.tile([DH, Q], BF16, tag=f"quh{h}", name=f"quh{h}")
             for h in range(H)]
    qv_h = [const.tile([DH, Q], FP32, tag=f"qvh{h}", name=f"qvh{h}")
            for h in range(H)]
    for dt in range(CB):
        ps = psum_big.tile([128, 1024], FP32, tag="big", name="ps_qp")
        for cb in range(CB):
            nc.tensor.matmul(
                ps[:, :Q], wslice(cb, WB_WQ, dt),
                wb_t[cb][:, WB_QRY:WB_QRY + Q],
                start=(cb == 0), stop=(cb == CB - 1))
        qu = work.tile([128, Q], FP32, tag="qu", name="qu")
        nc.vector.tensor_scalar_add(
            out=qu, in0=ps[:, :Q], scalar1=cols_t[:, 2 + dt:3 + dt])
        qv = work.tile([128, Q], FP32, tag="qv", name="qv")
        nc.vector.tensor_scalar_add(
            out=qv, in0=ps[:, :Q], scalar1=cols_t[:, 4 + dt:5 + dt])
        for hh in range(4):
            h = dt * 4 + hh
            nc.vector.tensor_copy(out=qu_hb[h], in_=qu[hh * DH:(hh + 1) * DH, :])
            nc.vector.tensor_copy(out=qv_h[h], in_=qv[hh * DH:(hh + 1) * DH, :])

    # v_proj natural [k, d'] + ones column per head -> v_aug [128, H*(DH+1)]
    ones_1 = const.tile([1, D], FP32, tag="ones", name="ones")
    nc.vector.memset(ones_1, 1.0)
    v_aug = []
    for kt in range(KT):
        ps = psum_out.tile([128, D], FP32, tag="pot", name="ps_v")
        for cb in range(CB):
            nc.tensor.matmul(
                ps, wb_t[cb][:, WB_VAL + kt * 128: WB_VAL + (kt + 1) * 128],
                wb_t[cb][:, WB_WV:WB_WV + D],
                start=(cb == 0), stop=False)
        nc.tensor.matmul(ps, ones_1[:, :128], bv_row, start=False, stop=True)
        va = const.tile([128, H, DH + 1], BF16, tag=f"va{kt}", name=f"va{kt}")
        nc.vector.memset(va, 1.0)
        nc.vector.tensor_copy(
            out=va[:, :, 0:DH], in_=ps.rearrange("p (h d) -> p h d", h=H))
        v_aug.append(va)

    # T_bf[cb][128, q, h] : T[:, q, h] = Wr_h^T @ qv_h[q]
    T_bf = [const.tile([128, Q, H], BF16, tag=f"T{cb}", name=f"Tbf{cb}")
            for cb in range(CB)]
    for h in range(H):
        for cb in range(CB):
            ps = psum_out.tile([128, Q], FP32, tag="po", name="ps_T")
            nc.tensor.matmul(
                ps, wrh_t[:, h * D + cb * 128: h * D + (cb + 1) * 128],
                qv_h[h], start=True, stop=True)
            nc.vector.tensor_copy(out=T_bf[cb][:, :, h], in_=ps)

    # ---------------- scores PSUM + A_C sweep ----------------
    # per k-tile: [128, 1024] f32 (2 banks); cols 8q+h used for pair q.
    # The h==0 matmul of each (kt, 64-pair bank region) opens that psum
    # accumulation group; the pair loop's final B_D matmul closes it.
    scores = [psum_big.tile([128, 1024], FP32, tag="big", name=f"scores{kt}")
              for kt in range(KT)]
    sc_v = [scores[kt][:, :Q * H].rearrange("p (q h) -> p q h", h=H)
            for kt in range(KT)]
    for kt in range(KT):
        for h in range(H):
            for r0, r1 in ((0, 64), (64, Q)):
                nc.tensor.matmul(
                    sc_v[kt][:, r0:r1, h],
                    kp_h[h][:, kt * 128:(kt + 1) * 128],
                    qu_hb[h][:, r0:r1],
                    start=(h == 0), stop=False)

    # ---------------- pos stream + B_D matmuls ----------------
    exp_sb = [const.tile([128, H, Q], BF16, tag=f"exp{kt}", name=f"exp{kt}")
              for kt in range(KT)]
    ex_v = [exp_sb[kt].rearrange("p h q -> p q h") for kt in range(KT)]

    def do_exp(r0, r1):
        for kt in range(KT):
            nc.scalar.activation(
                out=ex_v[kt][:, r0:r1, :], in_=sc_v[kt][:, r0:r1, :],
                func=mybir.ActivationFunctionType.Exp,
                bias=cols_t[:, 6 + kt:7 + kt], scale=float(SCALE))

    GSZ = PG * CB * L   # bf16 elems per partition per group
    for g in range(NG):
        pg = pos_pool.tile([128, GSZ], BF16, tag="posg", name=f"pos{g}")
        nc.sync.dma_start(out=pg, in_=posT[:, g * GSZ:(g + 1) * GSZ])
        for i in range(PG):
            p = g * PG + i
            for cb in range(CB):
                base = i * CB * L + cb * L
                for kt in range(KT):
                    stop = (cb == CB - 1) and (p in (63, Q - 1))
                    nc.tensor.matmul(
                        scores[kt][:, p * H:(p + 1) * H],
                        pg[:, base + kt * 128: base + kt * 128 + 128],
                        T_bf[cb][:, p, :],
                        start=False, stop=stop)
        if g * PG + PG - 1 == 63:
            do_exp(0, 64)   # bank-A regions closed; overlap exp with stream
    do_exp(64, Q)

    # ---------------- output matmuls + normalize ----------------
    out_sb = const.tile([Q, D], FP32, tag="osb", name="osb")
    for h in range(H):
        po = psum_out.tile([DH + 1, Q], FP32, tag="po", name="po")
        for kt in range(KT):
            nc.tensor.matmul(
                po, v_aug[kt][:, h, :], exp_sb[kt][:, h, :],
                start=(kt == 0), stop=(kt == KT - 1))
        tmp = work.tile([DH + 1, Q], FP32, tag="otmp", name="otmp")
        nc.vector.tensor_copy(out=tmp, in_=po)
        pot = psum_out.tile([Q, DH + 1], FP32, tag="pot", name="pot")
        nc.tensor.matmul(
            pot, tmp, ident_f[:DH + 1, :DH + 1],
            is_transpose=True, start=True, stop=True)
        rec = work.tile([Q, 1], FP32, tag="rec", name="rec")
        nc.vector.reciprocal(out=rec, in_=pot[:, DH:DH + 1])
        nc.vector.tensor_scalar_mul(
            out=out_sb[:, h * DH:(h + 1) * DH], in0=pot[:, 0:DH], scalar1=rec)

    nc.sync.dma_start(out=out, in_=out_sb)
    ctx.close()


def build_program():
    nc = bacc.Bacc(
        "TRN2", target_bir_lowering=False, debug=False,
        num_devices=NCORES)
    ins = {
        "posT": nc.dram_tensor(
            "posT", [128, Q * CB * L], BF16, kind="ExternalInput").ap(),
        "wb": nc.dram_tensor("wb", [D, WBC], FP32, kind="ExternalInput").ap(),
        "wrh": nc.dram_tensor(
            "wrh", [DH, H * D], FP32, kind="ExternalInput").ap(),
        "cols": nc.dram_tensor(
            "cols", [128, 12], FP32, kind="ExternalInput").ap(),
        "bvrow": nc.dram_tensor(
            "bvrow", [1, D], FP32, kind="ExternalInput").ap(),
    }
    outs = {
        "out": nc.dram_tensor("out", [Q, D], FP32, kind="ExternalOutput").ap(),
    }
    with tile.TileContext(nc) as tc:
        build_kernel_body(tc, outs, ins)
    nc.compile()
    return nc


def shard_inputs(inputs):
    """Full inputs -> list of 8 per-core input dicts (numpy, contiguous).

    All transposes/casts/packing happen here so the device kernel streams
    every tensor in its natural consumption order.
    """
    import ml_dtypes
    bf = ml_dtypes.bfloat16
    f32 = lambda a: np.ascontiguousarray(np.asarray(a), dtype=np.float32)
    pos = np.asarray(inputs["pos"], dtype=np.float32)
    key, query, value = f32(inputs["key"]), f32(inputs["query"]), f32(inputs["value"])
    mask = f32(inputs["key_mask"])
    Wk, Wq, Wv, Wr = (f32(inputs[k]) for k in ("Wk", "Wq", "Wv", "Wr"))
    bk, bq, bv = f32(inputs["bk"]), f32(inputs["bq"]), f32(inputs["bv"])
    u, v = f32(inputs["u"]), f32(inputs["v"])

    cols = np.zeros((128, 12), np.float32)
    uu, vv = bq + u.reshape(-1), bq + v.reshape(-1)
    cols[:, 0], cols[:, 1] = bk[:128], bk[128:]
    cols[:, 2], cols[:, 3] = uu[:128], uu[128:]
    cols[:, 4], cols[:, 5] = vv[:128], vv[128:]
    wrh = np.ascontiguousarray(
        Wr.reshape(H, DH, D).transpose(1, 0, 2).reshape(DH, H * D))
    bvrow = np.ascontiguousarray(bv.reshape(1, D))

    in_maps = []
    wb_b = {}
    for c in range(NCORES):
        b, q0 = c // 4, (c % 4) * Q
        colsb = cols.copy()
        mb = (mask[b] - 1.0) * 1e15
        colsb[:, 6], colsb[:, 7], colsb[:, 8] = mb[:128], mb[128:256], mb[256:]
        if b not in wb_b:
            wb_b[b] = np.ascontiguousarray(np.concatenate(
                [Wk.T, Wq.T, Wv.T, key[b].T, value[b].T], axis=1))
        wbm = np.ascontiguousarray(np.concatenate(
            [wb_b[b], query[b, q0:q0 + Q].T], axis=1))
        pb = pos[b, q0:q0 + Q].astype(bf)
        pr = np.ascontiguousarray(
            pb.reshape(Q, L, CB, 128).transpose(3, 0, 2, 1)
        ).reshape(128, Q * CB * L)
        in_maps.append({
            "posT": pr, "wb": wbm, "wrh": wrh, "cols": colsb, "bvrow": bvrow,
        })
    return in_maps


_CACHED = {}


def kernel(**inputs):
    from concourse.bass_utils import run_bass_kernel_spmd

    if "nc" not in _CACHED:
        _CACHED["nc"] = build_program()
    nc = _CACHED["nc"]
    in_maps = shard_inputs(inputs)
    res = run_bass_kernel_spmd(nc, in_maps, core_ids=list(range(NCORES)))
    out = np.zeros((B, L, D), dtype=np.float32)
    for c in range(NCORES):
        b, q0 = c // 4, (c % 4) * Q
        out[b, q0:q0 + Q] = res.results[c]["out"]
    return out


# revision 4
# speedup vs baseline: 2.6260x; 2.6260x over previous
"""Trainium2 Bass kernel for relative-position multi-head attention.

Shapes (hardcoded): B=2, L=384, D=256, H=8, DH=32.
Sharding: 8 cores; core c handles batch b=c//4, query rows [(c%4)*96, +96).
Pure data-parallel SPMD - no collectives.

Math (per batch b, query q):
  q/k/v projections: x @ W.T + bias
  A_C[h,k] = (q_h+u_h) . k_h[k]
  B_D[h,k] = (q_h+v_h) . (Wr_h @ pos[q,k] + br_h)
           = (Wr_h^T (q_h+v_h)) . pos[q,k]   + const(h,q)   [br term is
             k-independent -> cancels in softmax -> dropped]
  score    = (A_C + B_D)/sqrt(DH) - (1-mask[k])*1e15
  out      = softmax_k(score) @ v

Key restructurings:
  * r = pos @ Wr.T (38 GFLOP) is never materialized; instead
    T[q] = Wr^T-blockdiag @ (q+v)  (a [256,8] matrix per query) and
    B_D = posT @ T  (1.2 GFLOP).
  * ALL layout work happens on the host in shard_inputs: pos arrives as
    bf16 [128(d%128), Q, 2(d//128), L] (partition-major, so each pos DMA
    is a clean per-partition-contiguous stream), and every weight arrives
    pre-transposed/packed. The device runs only: projections, the 576
    B_D matmuls (6 per query pair: 2 d-blocks x 3 k-tiles, each
    LDW[128x128 bf16] + MM[N=8]), fused exp, and the output contraction.
  * scores live in PSUM as [k-partitions, (pair,h)-free]; softmax over k
    (partitions) uses exp on ACT + a ones-column appended to v_proj so the
    softmax denominator falls out of the output matmul for free. A_C is
    folded into the same PSUM accumulation (opens the group), B_D closes.
"""

import sys

for _p in ("/opt/trn_rl_repo", "/root/.axon_site/_ro/trn_rl_repo"):
    if _p not in sys.path:
        sys.path.append(_p)

import numpy as np

import concourse.bass as bass
import concourse.mybir as mybir
import concourse.tile as tile
from concourse import bacc
from concourse.masks import make_identity

FP32 = mybir.dt.float32
BF16 = mybir.dt.bfloat16

B, L, D, H = 2, 384, 256, 8
DH = D // H            # 32
Q = 96                 # queries per core
KT = L // 128          # 3 k-tiles
CB = D // 128          # 2 contraction blocks
NCORES = 8
SCALE = 1.0 / np.sqrt(DH)
PG = 8                 # pairs per pos DMA group
NG = Q // PG           # 12 groups

# column layout of the host-packed weight tensor "wb" [D, WBC]
WB_WK, WB_WQ, WB_WV = 0, D, 2 * D      # Wk.T / Wq.T / Wv.T   [D, D]
WB_KEY, WB_VAL = 3 * D, 3 * D + L      # key.T / value.T      [D, L]
WB_QRY = 3 * D + 2 * L                 # query.T              [D, Q]
WBC = 3 * D + 2 * L + Q                # 1632

# "cols" [128, 12] f32 per-partition columns:
#   0,1 = bk | 2,3 = bq+u | 4,5 = bq+v | 6,7,8 = (mask-1)*1e15 per k-tile


def build_kernel_body(tc, outs, ins):
    """Emit the per-core program. outs/ins are dicts of DRAM APs."""
    from contextlib import ExitStack
    ctx = ExitStack()
    pool = lambda **kw: ctx.enter_context(tc.tile_pool(**kw))
    nc = tc.nc
    posT = ins["posT"]      # [128, Q*CB*L] bf16  (p, q, cb, k)
    wb = ins["wb"]          # [D, WBC] f32
    wrh = ins["wrh"]        # [DH, H*D] f32 : wrh[i, h*D+d] = Wr[h*DH+i, d]
    cols = ins["cols"]      # [128, 12] f32
    bvrow = ins["bvrow"]    # [1, D] f32
    out = outs["out"]       # [Q, D] f32

    const = pool(name="const", bufs=1)
    work = pool(name="work", bufs=2)
    pos_pool = pool(name="pos", bufs=4)
    psum_big = pool(name="psum_big", bufs=3, space="PSUM")
    psum_out = pool(name="psum_out", bufs=1, space="PSUM")

    # ---------------- setup loads (Act ring; pos stream owns sync) ------
    wb_t = [const.tile([128, WBC], FP32, tag=f"wb{cb}", name=f"wb{cb}")
            for cb in range(CB)]
    for cb in range(CB):
        nc.scalar.dma_start(out=wb_t[cb], in_=wb[cb * 128:(cb + 1) * 128, :])
    wrh_t = const.tile([DH, H * D], FP32, tag="wrh", name="wrh")
    nc.scalar.dma_start(out=wrh_t, in_=wrh)
    cols_t = const.tile([128, 12], FP32, tag="cols", name="cols")
    nc.scalar.dma_start(out=cols_t, in_=cols)
    bv_row = const.tile([1, D], FP32, tag="bvrow", name="bvrow")
    nc.scalar.dma_start(out=bv_row, in_=bvrow)

    ident_f = const.tile([128, 128], FP32, tag="idf", name="idf")
    make_identity(nc, ident_f)

    def wslice(cb, base, j):  # [128, 128] lhsT slice of the packed weights
        return wb_t[cb][:, base + j * 128: base + (j + 1) * 128]

    # ---------------- projections ----------------
    # k_projT [d', k] bf16 (+bk), kept as 2 full d'-chunk tiles
    kpT_b = [const.tile([128, L], BF16, tag=f"kpb{dt}", name=f"kpb{dt}")
             for dt in range(CB)]
    for dt in range(CB):
        ps = psum_big.tile([128, 1024], FP32, tag="big", name="ps_kp")
        for cb in range(CB):
            nc.tensor.matmul(
                ps[:, :L], wslice(cb, WB_WK, dt),
                wb_t[cb][:, WB_KEY:WB_KEY + L],
                start=(cb == 0), stop=(cb == CB - 1))
        nc.vector.tensor_scalar_add(
            out=kpT_b[dt], in0=ps[:, :L], scalar1=cols_t[:, 0 + dt:1 + dt])

    # q_projT [d', q]; qu = +bq+u as block-diag QU [d', (q,h)] bf16 (A_C
    # rhs; zero except head-h block), qv = +bq+v per-head [32, q] (T rhs)
    QU = [const.tile([128, Q, H], BF16, tag=f"QU{dt}", name=f"QU{dt}")
          for dt in range(CB)]
    qv_h = [const.tile([DH, Q], FP32, tag=f"qvh{h}", name=f"qvh{h}")
            for h in range(H)]
    for dt in range(CB):
        ps = psum_big.tile([128, 1024], FP32, tag="big", name="ps_qp")
        for cb in range(CB):
            nc.tensor.matmul(
                ps[:, :Q], wslice(cb, WB_WQ, dt),
                wb_t[cb][:, WB_QRY:WB_QRY + Q],
                start=(cb == 0), stop=(cb == CB - 1))
        qu = work.tile([128, Q], FP32, tag="qu", name="qu")
        nc.vector.tensor_scalar_add(
            out=qu, in0=ps[:, :Q], scalar1=cols_t[:, 2 + dt:3 + dt])
        qv = work.tile([128, Q], FP32, tag="qv", name="qv")
        nc.vector.tensor_scalar_add(
            out=qv, in0=ps[:, :Q], scalar1=cols_t[:, 4 + dt:5 + dt])
        nc.vector.memset(QU[dt], 0.0)
        for hh in range(4):
            h = dt * 4 + hh
            nc.vector.tensor_copy(
                out=QU[dt][hh * DH:(hh + 1) * DH, :, h],
                in_=qu[hh * DH:(hh + 1) * DH, :])
            nc.vector.tensor_copy(out=qv_h[h], in_=qv[hh * DH:(hh + 1) * DH, :])

    # v_proj natural [k, d'] + ones column per head -> v_aug [128, H*(DH+1)]
    ones_1 = const.tile([1, D], FP32, tag="ones", name="ones")
    nc.vector.memset(ones_1, 1.0)
    v_aug = []
    for kt in range(KT):
        ps = psum_out.tile([128, D], FP32, tag="pot", name="ps_v")
        for cb in range(CB):
            nc.tensor.matmul(
                ps, wb_t[cb][:, WB_VAL + kt * 128: WB_VAL + (kt + 1) * 128],
                wb_t[cb][:, WB_WV:WB_WV + D],
                start=(cb == 0), stop=False)
        nc.tensor.matmul(ps, ones_1[:, :128], bv_row, start=False, stop=True)
        va = const.tile([128, H, DH + 1], BF16, tag=f"va{kt}", name=f"va{kt}")
        nc.vector.memset(va, 1.0)
        nc.vector.tensor_copy(
            out=va[:, :, 0:DH], in_=ps.rearrange("p (h d) -> p h d", h=H))
        v_aug.append(va)

    # T_bf[cb][128, q, h] : T[:, q, h] = Wr_h^T @ qv_h[q]
    T_bf = [const.tile([128, Q, H], BF16, tag=f"T{cb}", name=f"Tbf{cb}")
            for cb in range(CB)]
    for h in range(H):
        for cb in range(CB):
            ps = psum_out.tile([128, Q], FP32, tag="po", name="ps_T")
            nc.tensor.matmul(
                ps, wrh_t[:, h * D + cb * 128: h * D + (cb + 1) * 128],
                qv_h[h], start=True, stop=True)
            nc.vector.tensor_copy(out=T_bf[cb][:, :, h], in_=ps)

    # ---------------- scores PSUM + A_C sweep ----------------
    # per k-tile: [128, 1024] f32 (2 banks); cols 8q+h used for pair q.
    # A_C as 2 chained full-width matmuls per (kt, bank region): contraction
    # over all d'=256 against the block-diag QU. The cb==0 matmul writes the
    # region contiguously with start=True (clean whole-bank psum open); the
    # pair loop's final B_D matmul closes the region's accumulation group.
    scores = [psum_big.tile([128, 1024], FP32, tag="big", name=f"scores{kt}")
              for kt in range(KT)]
    sc_v = [scores[kt][:, :Q * H].rearrange("p (q h) -> p q h", h=H)
            for kt in range(KT)]
    qu_f = [QU[cb].rearrange("p q h -> p (q h)") for cb in range(CB)]
    for kt in range(KT):
        for c0, c1 in ((0, 512), (512, Q * H)):
            for cb in range(CB):
                nc.tensor.matmul(
                    scores[kt][:, c0:c1],
                    kpT_b[cb][:, kt * 128:(kt + 1) * 128],
                    qu_f[cb][:, c0:c1],
                    start=(cb == 0), stop=False)

    # ---------------- pos stream + B_D matmuls ----------------
    exp_sb = [const.tile([128, H, Q], BF16, tag=f"exp{kt}", name=f"exp{kt}")
              for kt in range(KT)]
    ex_v = [exp_sb[kt].rearrange("p h q -> p q h") for kt in range(KT)]

    def do_exp(r0, r1):
        for kt in range(KT):
            nc.scalar.activation(
                out=ex_v[kt][:, r0:r1, :], in_=sc_v[kt][:, r0:r1, :],
                func=mybir.ActivationFunctionType.Exp,
                bias=cols_t[:, 6 + kt:7 + kt], scale=float(SCALE))

    GSZ = PG * CB * L   # bf16 elems per partition per group
    for g in range(NG):
        pg = pos_pool.tile([128, GSZ], BF16, tag="posg", name=f"pos{g}")
        nc.sync.dma_start(out=pg, in_=posT[:, g * GSZ:(g + 1) * GSZ])
        for i in range(PG):
            p = g * PG + i
            for cb in range(CB):
                base = i * CB * L + cb * L
                for kt in range(KT):
                    stop = (cb == CB - 1) and (p in (63, Q - 1))
                    nc.tensor.matmul(
                        scores[kt][:, p * H:(p + 1) * H],
                        pg[:, base + kt * 128: base + kt * 128 + 128],
                        T_bf[cb][:, p, :],
                        start=False, stop=stop)
        if g * PG + PG - 1 == 63:
            do_exp(0, 64)   # bank-A regions closed; overlap exp with stream
    do_exp(64, Q)

    # ---------------- output matmuls + normalize ----------------
    out_sb = const.tile([Q, D], FP32, tag="osb", name="osb")
    for h in range(H):
        po = psum_out.tile([DH + 1, Q], FP32, tag="po", name="po")
        for kt in range(KT):
            nc.tensor.matmul(
                po, v_aug[kt][:, h, :], exp_sb[kt][:, h, :],
                start=(kt == 0), stop=(kt == KT - 1))
        tmp = work.tile([DH + 1, Q], FP32, tag="otmp", name="otmp")
        nc.vector.tensor_copy(out=tmp, in_=po)
        pot = psum_out.tile([Q, DH + 1], FP32, tag="pot", name="pot")
        nc.tensor.matmul(
            pot, tmp, ident_f[:DH + 1, :DH + 1],
            is_transpose=True, start=True, stop=True)
        rec = work.tile([Q, 1], FP32, tag="rec", name="rec")
        nc.vector.reciprocal(out=rec, in_=pot[:, DH:DH + 1])
        nc.vector.tensor_scalar_mul(
            out=out_sb[:, h * DH:(h + 1) * DH], in0=pot[:, 0:DH], scalar1=rec)

    nc.sync.dma_start(out=out, in_=out_sb)
    ctx.close()


def build_program():
    nc = bacc.Bacc(
        "TRN2", target_bir_lowering=False, debug=False,
        num_devices=NCORES)
    ins = {
        "posT": nc.dram_tensor(
            "posT", [128, Q * CB * L], BF16, kind="ExternalInput").ap(),
        "wb": nc.dram_tensor("wb", [D, WBC], FP32, kind="ExternalInput").ap(),
        "wrh": nc.dram_tensor(
            "wrh", [DH, H * D], FP32, kind="ExternalInput").ap(),
        "cols": nc.dram_tensor(
            "cols", [128, 12], FP32, kind="ExternalInput").ap(),
        "bvrow": nc.dram_tensor(
            "bvrow", [1, D], FP32, kind="ExternalInput").ap(),
    }
    outs = {
        "out": nc.dram_tensor("out", [Q, D], FP32, kind="ExternalOutput").ap(),
    }
    with tile.TileContext(nc) as tc:
        build_kernel_body(tc, outs, ins)
    nc.compile()
    return nc


def shard_inputs(inputs):
    """Full inputs -> list of 8 per-core input dicts (numpy, contiguous).

    All transposes/casts/packing happen here so the device kernel streams
    every tensor in its natural consumption order.
    """
    import ml_dtypes
    bf = ml_dtypes.bfloat16
    f32 = lambda a: np.ascontiguousarray(np.asarray(a), dtype=np.float32)
    pos = np.asarray(inputs["pos"], dtype=np.float32)
    key, query, value = f32(inputs["key"]), f32(inputs["query"]), f32(inputs["value"])
    mask = f32(inputs["key_mask"])
    Wk, Wq, Wv, Wr = (f32(inputs[k]) for k in ("Wk", "Wq", "Wv", "Wr"))
    bk, bq, bv = f32(inputs["bk"]), f32(inputs["bq"]), f32(inputs["bv"])
    u, v = f32(inputs["u"]), f32(inputs["v"])

    cols = np.zeros((128, 12), np.float32)
    uu, vv = bq + u.reshape(-1), bq + v.reshape(-1)
    cols[:, 0], cols[:, 1] = bk[:128], bk[128:]
    cols[:, 2], cols[:, 3] = uu[:128], uu[128:]
    cols[:, 4], cols[:, 5] = vv[:128], vv[128:]
    wrh = np.ascontiguousarray(
        Wr.reshape(H, DH, D).transpose(1, 0, 2).reshape(DH, H * D))
    bvrow = np.ascontiguousarray(bv.reshape(1, D))

    in_maps = []
    wb_b = {}
    for c in range(NCORES):
        b, q0 = c // 4, (c % 4) * Q
        colsb = cols.copy()
        mb = (mask[b] - 1.0) * 1e15
        colsb[:, 6], colsb[:, 7], colsb[:, 8] = mb[:128], mb[128:256], mb[256:]
        if b not in wb_b:
            wb_b[b] = np.ascontiguousarray(np.concatenate(
                [Wk.T, Wq.T, Wv.T, key[b].T, value[b].T], axis=1))
        wbm = np.ascontiguousarray(np.concatenate(
            [wb_b[b], query[b, q0:q0 + Q].T], axis=1))
        pb = pos[b, q0:q0 + Q].astype(bf)
        pr = np.ascontiguousarray(
            pb.reshape(Q, L, CB, 128).transpose(3, 0, 2, 1)
        ).reshape(128, Q * CB * L)
        in_maps.append({
            "posT": pr, "wb": wbm, "wrh": wrh, "cols": colsb, "bvrow": bvrow,
        })
    return in_maps


_CACHED = {}


def kernel(**inputs):
    from concourse.bass_utils import run_bass_kernel_spmd

    if "nc" not in _CACHED:
        _CACHED["nc"] = build_program()
    nc = _CACHED["nc"]
    in_maps = shard_inputs(inputs)
    res = run_bass_kernel_spmd(nc, in_maps, core_ids=list(range(NCORES)))
    out = np.zeros((B, L, D), dtype=np.float32)
    for c in range(NCORES):
        b, q0 = c // 4, (c % 4) * Q
        out[b, q0:q0 + Q] = res.results[c]["out"]
    return out


# revision 5
# speedup vs baseline: 3.4644x; 1.3192x over previous
"""Trainium2 Bass kernel for relative-position multi-head attention.

Shapes (hardcoded): B=2, L=384, D=256, H=8, DH=32.
Sharding: 8 cores; core c handles batch b=c//4, query rows [(c%4)*96, +96).
Pure data-parallel SPMD - no collectives.

Math (per batch b, query q):
  q/k/v projections: x @ W.T + bias
  A_C[h,k] = (q_h+u_h) . k_h[k]
  B_D[h,k] = (q_h+v_h) . (Wr_h @ pos[q,k] + br_h)
           = (Wr_h^T (q_h+v_h)) . pos[q,k]   + const(h,q)   [br term is
             k-independent -> cancels in softmax -> dropped]
  score    = (A_C + B_D)/sqrt(DH) - (1-mask[k])*1e15
  out      = softmax_k(score) @ v

Key restructurings:
  * r = pos @ Wr.T (38 GFLOP) is never materialized; instead
    T[q] = Wr^T-blockdiag @ (q+v)  (a [256,8] matrix per query) and
    B_D = posT @ T  (1.2 GFLOP).
  * ALL layout work happens on the host in shard_inputs: pos arrives as
    bf16 [128(d%128), Q, 2(d//128), L] (partition-major, each pos DMA is a
    per-partition-contiguous stream), weights arrive bf16 pre-transposed.
  * scores live in PSUM as [k-partitions, (pair,h)-free]. A_C and T are
    computed with block-diagonal (q+u)/(q+v) operands so each is a handful
    of full-width matmuls; A_C's cb==0 matmul opens each psum bank
    contiguously, the pair loop's last B_D matmul closes it.
  * softmax denominator via a ones-column appended to v_proj; exp on ACT
    (contiguous in/out) with mask bias + 1/sqrt(dh) scale fused; the
    output matmul consumes exp as lhsT, yielding out^T per head directly
    (no transposes anywhere in the kernel).
"""

import sys

for _p in ("/opt/trn_rl_repo", "/root/.axon_site/_ro/trn_rl_repo"):
    if _p not in sys.path:
        sys.path.append(_p)

import numpy as np

import concourse.bass as bass
import concourse.mybir as mybir
import concourse.tile as tile
from concourse import bacc

FP32 = mybir.dt.float32
BF16 = mybir.dt.bfloat16

B, L, D, H = 2, 384, 256, 8
DH = D // H            # 32
Q = 96                 # queries per core
KT = L // 128          # 3 k-tiles
CB = D // 128          # 2 contraction blocks
NCORES = 8
SCALE = 1.0 / np.sqrt(DH)
PG = 8                 # pairs per pos DMA group
NG = Q // PG           # 12 groups

# column layout of the host-packed bf16 weight tensor "wb" [D, WBC]
WB_WK, WB_WQ, WB_WV, WB_WR = 0, D, 2 * D, 3 * D   # Wk.T/Wq.T/Wv.T/Wr
WB_KEY, WB_VAL = 4 * D, 4 * D + L                  # key.T / value.T [D, L]
WB_QRY = 4 * D + 2 * L                             # query.T [D, Q]
WBC = 4 * D + 2 * L + Q                            # 1888

# "cols" [128, 12] f32 per-partition columns:
#   0,1 = bk | 2,3 = bq+u | 4,5 = bq+v | 6,7,8 = (mask-1)*1e15 per k-tile


def build_kernel_body(tc, outs, ins):
    """Emit the per-core program. outs/ins are dicts of DRAM APs."""
    from contextlib import ExitStack
    ctx = ExitStack()
    pool = lambda **kw: ctx.enter_context(tc.tile_pool(**kw))
    nc = tc.nc
    posT = ins["posT"]      # [128, Q*CB*L] bf16  (p, q, cb, k)
    wb = ins["wb"]          # [D, WBC] bf16
    cols = ins["cols"]      # [128, 12] f32
    bvrow = ins["bvrow"]    # [1, D] f32
    out = outs["out"]       # [Q, D] f32

    const = pool(name="const", bufs=1)
    pos_pool = pool(name="pos", bufs=6)
    psum_big = pool(name="psum_big", bufs=3, space="PSUM")
    psum_out = pool(name="psum_out", bufs=2, space="PSUM")

    # ---------------- setup loads (Act ring; pos stream owns sync) ------
    wb_t = [const.tile([128, WBC], BF16, tag=f"wb{cb}", name=f"wb{cb}")
            for cb in range(CB)]
    for cb in range(CB):
        nc.scalar.dma_start(out=wb_t[cb], in_=wb[cb * 128:(cb + 1) * 128, :])
    cols_t = const.tile([128, 12], FP32, tag="cols", name="cols")
    nc.scalar.dma_start(out=cols_t, in_=cols)
    bv_row = const.tile([1, D], FP32, tag="bvrow", name="bvrow")
    nc.scalar.dma_start(out=bv_row, in_=bvrow)

    def wslice(cb, base, j):  # [128, 128] lhsT slice of the packed weights
        return wb_t[cb][:, base + j * 128: base + (j + 1) * 128]

    # ---------------- projections ----------------
    # k_projT [d', k] bf16 (+bk), kept as 2 full d'-chunk tiles
    kpT_b = [const.tile([128, L], BF16, tag=f"kpb{dt}", name=f"kpb{dt}")
             for dt in range(CB)]
    for dt in range(CB):
        ps = psum_big.tile([128, 1024], FP32, tag="big", name="ps_kp")
        for cb in range(CB):
            nc.tensor.matmul(
                ps[:, :L], wslice(cb, WB_WK, dt),
                wb_t[cb][:, WB_KEY:WB_KEY + L],
                start=(cb == 0), stop=(cb == CB - 1))
        nc.vector.tensor_scalar_add(
            out=kpT_b[dt], in0=ps[:, :L], scalar1=cols_t[:, 0 + dt:1 + dt])

    # q_projT [d', q]; block-diag QU = q+bq+u and QV = q+bq+v, bf16
    # [d', (q,h)] zero except each head's 32-row block (A_C rhs / T rhs)
    QU = [const.tile([128, Q * H], BF16, tag=f"QU{dt}", name=f"QU{dt}")
          for dt in range(CB)]
    QV = [const.tile([128, Q * H], BF16, tag=f"QV{dt}", name=f"QV{dt}")
          for dt in range(CB)]
    for dt in range(CB):
        nc.vector.memset(QU[dt], 0.0)
        nc.vector.memset(QV[dt], 0.0)
    qu_v = [QU[dt].rearrange("p (q h) -> p q h", h=H) for dt in range(CB)]
    qv_v = [QV[dt].rearrange("p (q h) -> p q h", h=H) for dt in range(CB)]
    for dt in range(CB):
        ps = psum_big.tile([128, 1024], FP32, tag="big", name="ps_qp")
        for cb in range(CB):
            nc.tensor.matmul(
                ps[:, :Q], wslice(cb, WB_WQ, dt),
                wb_t[cb][:, WB_QRY:WB_QRY + Q],
                start=(cb == 0), stop=(cb == CB - 1))
        for hh in range(4):
            h = dt * 4 + hh
            nc.vector.tensor_scalar_add(
                out=qu_v[dt][hh * DH:(hh + 1) * DH, :, h],
                in0=ps[hh * DH:(hh + 1) * DH, :Q],
                scalar1=cols_t[hh * DH:(hh + 1) * DH, 2 + dt:3 + dt])
            nc.vector.tensor_scalar_add(
                out=qv_v[dt][hh * DH:(hh + 1) * DH, :, h],
                in0=ps[hh * DH:(hh + 1) * DH, :Q],
                scalar1=cols_t[hh * DH:(hh + 1) * DH, 4 + dt:5 + dt])

    # T_bf[cb][d(128), (q,h)] = sum_d' Wr[d', cb*128+d] * QV[d', (q,h)]
    T_bf = [const.tile([128, Q * H], BF16, tag=f"T{cb}", name=f"Tbf{cb}")
            for cb in range(CB)]
    for cbo in range(CB):
        ps = psum_big.tile([128, 1024], FP32, tag="big", name="ps_T")
        for c0, c1 in ((0, 512), (512, Q * H)):
            for cb in range(CB):
                nc.tensor.matmul(
                    ps[:, c0:c1], wslice(cb, WB_WR, cbo), QV[cb][:, c0:c1],
                    start=(cb == 0), stop=(cb == CB - 1))
        nc.vector.tensor_copy(out=T_bf[cbo], in_=ps[:, :Q * H])

    # v_proj natural [k, d'] + ones column per head -> v_aug [128, H*(DH+1)]
    ones_1 = const.tile([1, D], FP32, tag="ones", name="ones")
    nc.vector.memset(ones_1, 1.0)
    v_aug = []
    for kt in range(KT):
        ps = psum_out.tile([128, D], FP32, tag="pot", name="ps_v")
        for cb in range(CB):
            nc.tensor.matmul(
                ps, wb_t[cb][:, WB_VAL + kt * 128: WB_VAL + (kt + 1) * 128],
                wb_t[cb][:, WB_WV:WB_WV + D],
                start=(cb == 0), stop=False)
        nc.tensor.matmul(ps, ones_1[:, :128], bv_row, start=False, stop=True)
        va = const.tile([128, H, DH + 1], BF16, tag=f"va{kt}", name=f"va{kt}")
        nc.vector.memset(va, 1.0)
        nc.vector.tensor_copy(
            out=va[:, :, 0:DH], in_=ps.rearrange("p (h d) -> p h d", h=H))
        v_aug.append(va)

    # ---------------- scores PSUM + A_C sweep ----------------
    # per k-tile: [128, 1024] f32 (2 banks); col 8q+h holds (pair q, head h).
    # A_C: 2 chained full-width matmuls per (kt, bank): contraction over all
    # d'=256 against block-diag QU. cb==0 writes the bank contiguously with
    # start=True (clean whole-bank open); the last B_D matmul closes it.
    scores = [psum_big.tile([128, 1024], FP32, tag="big", name=f"scores{kt}")
              for kt in range(KT)]
    for kt in range(KT):
        for c0, c1 in ((0, 512), (512, Q * H)):
            for cb in range(CB):
                nc.tensor.matmul(
                    scores[kt][:, c0:c1],
                    kpT_b[cb][:, kt * 128:(kt + 1) * 128],
                    QU[cb][:, c0:c1],
                    start=(cb == 0), stop=False)

    # ---------------- pos stream + B_D matmuls ----------------
    exp_sb = [const.tile([128, Q * H], BF16, tag=f"exp{kt}", name=f"exp{kt}")
              for kt in range(KT)]

    def do_exp(c0, c1):
        for kt in range(KT):
            nc.scalar.activation(
                out=exp_sb[kt][:, c0:c1], in_=scores[kt][:, c0:c1],
                func=mybir.ActivationFunctionType.Exp,
                bias=cols_t[:, 6 + kt:7 + kt], scale=float(SCALE))

    GSZ = PG * CB * L   # bf16 elems per partition per group
    for g in range(NG):
        pg = pos_pool.tile([128, GSZ], BF16, tag="posg", name=f"pos{g}")
        nc.sync.dma_start(out=pg, in_=posT[:, g * GSZ:(g + 1) * GSZ])
        for i in range(PG):
            p = g * PG + i
            for cb in range(CB):
                base = i * CB * L + cb * L
                for kt in range(KT):
                    stop = (cb == CB - 1) and (p in (63, Q - 1))
                    nc.tensor.matmul(
                        scores[kt][:, p * H:(p + 1) * H],
                        pg[:, base + kt * 128: base + kt * 128 + 128],
                        T_bf[cb][:, p * H:(p + 1) * H],
                        start=False, stop=stop)
        if g * PG + PG - 1 == 63:
            do_exp(0, 512)   # bank-A closed; overlap exp with the stream
    do_exp(512, Q * H)

    # ---------------- output matmuls + normalize ----------------
    # pot[q, j] = sum_k exp[k, (q,h)] * v_aug[k, (h,j)]  (exp as lhsT ->
    # out^T directly, no transposes); col DH = softmax denominator.
    out_sb = const.tile([Q, D], FP32, tag="osb", name="osb")
    ex_v = [exp_sb[kt].rearrange("p (q h) -> p q h", h=H) for kt in range(KT)]
    for h in range(H):
        pot = psum_out.tile([Q, DH + 1], FP32, tag="pot", name="pot")
        for kt in range(KT):
            nc.tensor.matmul(
                pot, ex_v[kt][:, :, h], v_aug[kt][:, h, :],
                start=(kt == 0), stop=(kt == KT - 1))
        rec = const.tile([Q, 1], FP32, tag=f"rec{h}", name=f"rec{h}")
        nc.vector.reciprocal(out=rec, in_=pot[:, DH:DH + 1])
        nc.vector.tensor_scalar_mul(
            out=out_sb[:, h * DH:(h + 1) * DH], in0=pot[:, 0:DH], scalar1=rec)

    nc.sync.dma_start(out=out, in_=out_sb)
    ctx.close()


def build_program():
    nc = bacc.Bacc(
        "TRN2", target_bir_lowering=False, debug=False,
        num_devices=NCORES)
    ins = {
        "posT": nc.dram_tensor(
            "posT", [128, Q * CB * L], BF16, kind="ExternalInput").ap(),
        "wb": nc.dram_tensor("wb", [D, WBC], BF16, kind="ExternalInput").ap(),
        "cols": nc.dram_tensor(
            "cols", [128, 12], FP32, kind="ExternalInput").ap(),
        "bvrow": nc.dram_tensor(
            "bvrow", [1, D], FP32, kind="ExternalInput").ap(),
    }
    outs = {
        "out": nc.dram_tensor("out", [Q, D], FP32, kind="ExternalOutput").ap(),
    }
    with tile.TileContext(nc) as tc:
        build_kernel_body(tc, outs, ins)
    nc.compile()
    return nc


def shard_inputs(inputs):
    """Full inputs -> list of 8 per-core input dicts (numpy, contiguous).

    All transposes/casts/packing happen here so the device kernel streams
    every tensor in its natural consumption order.
    """
    import ml_dtypes
    bf = ml_dtypes.bfloat16
    f32 = lambda a: np.ascontiguousarray(np.asarray(a), dtype=np.float32)
    pos = np.asarray(inputs["pos"], dtype=np.float32)
    key, query, value = f32(inputs["key"]), f32(inputs["query"]), f32(inputs["value"])
    mask = f32(inputs["key_mask"])
    Wk, Wq, Wv, Wr = (f32(inputs[k]) for k in ("Wk", "Wq", "Wv", "Wr"))
    bk, bq, bv = f32(inputs["bk"]), f32(inputs["bq"]), f32(inputs["bv"])
    u, v = f32(inputs["u"]), f32(inputs["v"])

    cols = np.zeros((128, 12), np.float32)
    uu, vv = bq + u.reshape(-1), bq + v.reshape(-1)
    cols[:, 0], cols[:, 1] = bk[:128], bk[128:]
    cols[:, 2], cols[:, 3] = uu[:128], uu[128:]
    cols[:, 4], cols[:, 5] = vv[:128], vv[128:]
    bvrow = np.ascontiguousarray(bv.reshape(1, D))

    in_maps = []
    wb_b = {}
    for c in range(NCORES):
        b, q0 = c // 4, (c % 4) * Q
        colsb = cols.copy()
        mb = (mask[b] - 1.0) * 1e15
        colsb[:, 6], colsb[:, 7], colsb[:, 8] = mb[:128], mb[128:256], mb[256:]
        if b not in wb_b:
            wb_b[b] = np.concatenate(
                [Wk.T, Wq.T, Wv.T, Wr, key[b].T, value[b].T],
                axis=1).astype(bf)
        wbm = np.ascontiguousarray(np.concatenate(
            [wb_b[b], query[b, q0:q0 + Q].T.astype(bf)], axis=1))
        pb = pos[b, q0:q0 + Q].astype(bf)
        pr = np.ascontiguousarray(
            pb.reshape(Q, L, CB, 128).transpose(3, 0, 2, 1)
        ).reshape(128, Q * CB * L)
        in_maps.append({
            "posT": pr, "wb": wbm, "cols": colsb, "bvrow": bvrow,
        })
    return in_maps


_CACHED = {}


def kernel(**inputs):
    from concourse.bass_utils import run_bass_kernel_spmd

    if "nc" not in _CACHED:
        _CACHED["nc"] = build_program()
    nc = _CACHED["nc"]
    in_maps = shard_inputs(inputs)
    res = run_bass_kernel_spmd(nc, in_maps, core_ids=list(range(NCORES)))
    out = np.zeros((B, L, D), dtype=np.float32)
    for c in range(NCORES):
        b, q0 = c // 4, (c % 4) * Q
        out[b, q0:q0 + Q] = res.results[c]["out"]
    return out


# revision 6
# speedup vs baseline: 3.8348x; 1.1069x over previous
"""Trainium2 Bass kernel for relative-position multi-head attention.

Shapes (hardcoded): B=2, L=384, D=256, H=8, DH=32.
Sharding: 8 cores; core c handles batch b=c//4, query rows [(c%4)*96, +96).
Pure data-parallel SPMD - no collectives.

Math (per batch b, query q):
  q/k/v projections: x @ W.T + bias
  A_C[h,k] = (q_h+u_h) . k_h[k]
  B_D[h,k] = (q_h+v_h) . (Wr_h @ pos[q,k] + br_h)
           = (Wr_h^T (q_h+v_h)) . pos[q,k]   + const(h,q)   [br term is
             k-independent -> cancels in softmax -> dropped]
  score    = (A_C + B_D)/sqrt(DH) - (1-mask[k])*1e15
  out      = softmax_k(score) @ v

Key restructurings:
  * r = pos @ Wr.T (38 GFLOP) is never materialized; instead
    T[q] = Wr^T-blockdiag @ (q+v)  (a [256,8] matrix per query) and
    B_D = posT @ T  (1.2 GFLOP).
  * ALL layout work happens on the host in shard_inputs: pos arrives as
    bf16 [128(d%128), Q, 2(d//128), L] (partition-major, each pos DMA is a
    per-partition-contiguous stream), weights arrive bf16 pre-transposed.
  * scores live in PSUM as [k-partitions, (pair,h)-free]. The B_D pair
    stream OPENS each psum bank (start=True on pair 0/64) and A_C -
    emitted after the stream - accumulates on top and CLOSES it, so B_D
    only waits on the short qv->T chain, never on A_C/k-proj setup.
  * A_C and T use block-diagonal (q+u)/(q+v) operands (a handful of
    full-width matmuls). QV is built on Vector (critical path to T);
    QU on GpSimd (slack until A_C at the end of the stream).
  * softmax denominator via a ones-column appended to v_proj; exp on ACT
    with mask bias + 1/sqrt(dh) scale fused; the output matmul consumes
    exp as contiguous lhsT yielding out^T per head directly (zero
    transposes anywhere in the kernel).
"""

import sys

for _p in ("/opt/trn_rl_repo", "/root/.axon_site/_ro/trn_rl_repo"):
    if _p not in sys.path:
        sys.path.append(_p)

import numpy as np

import concourse.bass as bass
import concourse.mybir as mybir
import concourse.tile as tile
from concourse import bacc

FP32 = mybir.dt.float32
BF16 = mybir.dt.bfloat16

B, L, D, H = 2, 384, 256, 8
DH = D // H            # 32
Q = 96                 # queries per core
KT = L // 128          # 3 k-tiles
CB = D // 128          # 2 contraction blocks
NCORES = 8
SCALE = 1.0 / np.sqrt(DH)
PG = 8                 # pairs per pos DMA group
NG = Q // PG           # 12 groups

# host-packed bf16 weight tensors:
#   wb1 [D, 608]:  Wq.T | Wr | query.T   (q-proj + T critical path)
W1_WQ, W1_WR, W1_QRY = 0, D, 2 * D
WBC1 = 2 * D + Q
#   wb2 [D, 1280]: Wk.T | Wv.T | key.T | value.T
W2_WK, W2_WV, W2_KEY, W2_VAL = 0, D, 2 * D, 2 * D + L
WBC2 = 2 * D + 2 * L

# "cols" [128, 12] f32 per-partition columns:
#   0,1 = bk | 2,3 = bq+u | 4,5 = bq+v | 6,7,8 = (mask-1)*1e15 per k-tile


def build_kernel_body(tc, outs, ins):
    """Emit the per-core program. outs/ins are dicts of DRAM APs."""
    from contextlib import ExitStack
    ctx = ExitStack()
    pool = lambda **kw: ctx.enter_context(tc.tile_pool(**kw))
    nc = tc.nc
    posT = ins["posT"]      # [128, Q*CB*L] bf16  (p, q, cb, k)
    wb1 = ins["wb1"]        # [D, WBC1] bf16
    wb2 = ins["wb2"]        # [D, WBC2] bf16
    cols = ins["cols"]      # [128, 12] f32
    bvb = ins["bvb"]        # [128, D] f32 (bv broadcast to 128 rows)
    out = outs["out"]       # [Q, D] f32

    const = pool(name="const", bufs=1)
    pos_pool = pool(name="pos", bufs=8)
    psum_big = pool(name="psum_big", bufs=3, space="PSUM")
    psum_out = pool(name="psum_out", bufs=2, space="PSUM")

    # ---------------- setup loads ----------------
    # critical path (cols, wb1) on the Act HWDGE ring; bulk (wb2, bvb) on
    # the GpSimd SWDGE ring; the pos stream owns the sync ring.
    cols_t = const.tile([128, 12], FP32, tag="cols", name="cols")
    nc.scalar.dma_start(out=cols_t, in_=cols)
    wb1_t = [const.tile([128, WBC1], BF16, tag=f"wb1_{cb}", name=f"wb1_{cb}")
             for cb in range(CB)]
    for cb in range(CB):
        nc.scalar.dma_start(out=wb1_t[cb], in_=wb1[cb * 128:(cb + 1) * 128, :])
    wb2_t = [const.tile([128, WBC2], BF16, tag=f"wb2_{cb}", name=f"wb2_{cb}")
             for cb in range(CB)]
    for cb in range(CB):
        nc.gpsimd.dma_start(out=wb2_t[cb], in_=wb2[cb * 128:(cb + 1) * 128, :])
    bvb_t = const.tile([128, D], FP32, tag="bvb", name="bvb")
    nc.gpsimd.dma_start(out=bvb_t, in_=bvb)

    # ---------------- q-projection -> QV (Vector) / QU (GpSimd) ---------
    # qp_sb[dt] [128, q] f32; QV/QU [d', (q,h)] bf16 block-diagonal
    qp_sb = [const.tile([128, Q], FP32, tag=f"qp{dt}", name=f"qp{dt}")
             for dt in range(CB)]
    QV = [const.tile([128, Q * H], BF16, tag=f"QV{dt}", name=f"QV{dt}")
          for dt in range(CB)]
    QU = [const.tile([128, Q * H], BF16, tag=f"QU{dt}", name=f"QU{dt}")
          for dt in range(CB)]
    qv_v = [QV[dt].rearrange("p (q h) -> p q h", h=H) for dt in range(CB)]
    qu_v = [QU[dt].rearrange("p (q h) -> p q h", h=H) for dt in range(CB)]
    for dt in range(CB):
        nc.vector.memset(QV[dt], 0.0)
        nc.gpsimd.memset(QU[dt], 0.0)
    for dt in range(CB):
        ps = psum_big.tile([128, 1024], FP32, tag="big", name="ps_qp")
        for cb in range(CB):
            nc.tensor.matmul(
                ps[:, :Q], wb1_t[cb][:, W1_WQ + dt * 128:W1_WQ + (dt + 1) * 128],
                wb1_t[cb][:, W1_QRY:W1_QRY + Q],
                start=(cb == 0), stop=(cb == CB - 1))
        nc.vector.tensor_copy(out=qp_sb[dt], in_=ps[:, :Q])
        for hh in range(4):
            h = dt * 4 + hh
            sl = slice(hh * DH, (hh + 1) * DH)
            nc.vector.tensor_scalar_add(
                out=qv_v[dt][sl, :, h], in0=qp_sb[dt][sl, :],
                scalar1=cols_t[sl, 4 + dt:5 + dt])
            nc.gpsimd.tensor_scalar_add(
                out=qu_v[dt][sl, :, h], in0=qp_sb[dt][sl, :],
                scalar1=cols_t[sl, 2 + dt:3 + dt])

    # ---------------- T matrices (B_D weights) ----------------
    # T_bf[cb][d(128), (q,h)] = sum_d' Wr[d', cb*128+d] * QV[d', (q,h)]
    T_bf = [const.tile([128, Q * H], BF16, tag=f"T{cb}", name=f"Tbf{cb}")
            for cb in range(CB)]
    for cbo in range(CB):
        ps = psum_big.tile([128, 1024], FP32, tag="big", name="ps_T")
        for c0, c1 in ((0, 512), (512, Q * H)):
            for cb in range(CB):
                nc.tensor.matmul(
                    ps[:, c0:c1],
                    wb1_t[cb][:, W1_WR + cbo * 128:W1_WR + (cbo + 1) * 128],
                    QV[cb][:, c0:c1],
                    start=(cb == 0), stop=(cb == CB - 1))
        nc.vector.tensor_copy(out=T_bf[cbo], in_=ps[:, :Q * H])

    # ---------------- k-projection (for the trailing A_C) ----------------
    kpT_b = [const.tile([128, L], BF16, tag=f"kpb{dt}", name=f"kpb{dt}")
             for dt in range(CB)]
    for dt in range(CB):
        ps = psum_big.tile([128, 1024], FP32, tag="big", name="ps_kp")
        for cb in range(CB):
            nc.tensor.matmul(
                ps[:, :L], wb2_t[cb][:, W2_WK + dt * 128:W2_WK + (dt + 1) * 128],
                wb2_t[cb][:, W2_KEY:W2_KEY + L],
                start=(cb == 0), stop=(cb == CB - 1))
        nc.vector.tensor_scalar_add(
            out=kpT_b[dt], in0=ps[:, :L], scalar1=cols_t[:, 0 + dt:1 + dt])

    # ---------------- v_proj + ones column -> v_aug ----------------
    v_aug = []
    for kt in range(KT):
        ps = psum_out.tile([128, D], FP32, tag="pot", name="ps_v")
        for cb in range(CB):
            nc.tensor.matmul(
                ps, wb2_t[cb][:, W2_VAL + kt * 128:W2_VAL + (kt + 1) * 128],
                wb2_t[cb][:, W2_WV:W2_WV + D],
                start=(cb == 0), stop=(cb == CB - 1))
        va = const.tile([128, H, DH + 1], BF16, tag=f"va{kt}", name=f"va{kt}")
        nc.vector.memset(va, 1.0)
        nc.vector.tensor_add(
            out=va[:, :, 0:DH], in0=ps.rearrange("p (h d) -> p h d", h=H),
            in1=bvb_t.rearrange("p (h d) -> p h d", h=H))
        v_aug.append(va)

    # ---------------- scores PSUM ----------------
    # per k-tile: [128, 1024] f32 (2 banks); col 8q+h holds (pair q, head h).
    # B_D pair 0 / pair 64 cb==0 opens each bank (start=True); A_C (below,
    # emitted mid/post stream) accumulates and closes it (stop=True).
    scores = [psum_big.tile([128, 1024], FP32, tag="big", name=f"scores{kt}")
              for kt in range(KT)]

    def do_ac(c0, c1):
        for kt in range(KT):
            for cb in range(CB):
                nc.tensor.matmul(
                    scores[kt][:, c0:c1],
                    kpT_b[cb][:, kt * 128:(kt + 1) * 128],
                    QU[cb][:, c0:c1],
                    start=False, stop=(cb == CB - 1))

    # exp_sb h-major [128, (h,q)] so output matmuls get contiguous lhsT
    exp_sb = [const.tile([128, H, Q], BF16, tag=f"exp{kt}", name=f"exp{kt}")
              for kt in range(KT)]
    ex_qh = [exp_sb[kt].rearrange("p h q -> p q h") for kt in range(KT)]
    sc_qh = [scores[kt][:, :Q * H].rearrange("p (q h) -> p q h", h=H)
             for kt in range(KT)]

    def do_exp(r0, r1):
        for kt in range(KT):
            nc.scalar.activation(
                out=ex_qh[kt][:, r0:r1, :], in_=sc_qh[kt][:, r0:r1, :],
                func=mybir.ActivationFunctionType.Exp,
                bias=cols_t[:, 6 + kt:7 + kt], scale=float(SCALE))

    # ---------------- pos stream + B_D matmuls ----------------
    GSZ = PG * CB * L   # bf16 elems per partition per group
    for g in range(NG):
        pg = pos_pool.tile([128, GSZ], BF16, tag="posg", name=f"pos{g}")
        nc.sync.dma_start(out=pg, in_=posT[:, g * GSZ:(g + 1) * GSZ])
        for i in range(PG):
            p = g * PG + i
            for cb in range(CB):
                base = i * CB * L + cb * L
                for kt in range(KT):
                    nc.tensor.matmul(
                        scores[kt][:, p * H:(p + 1) * H],
                        pg[:, base + kt * 128: base + kt * 128 + 128],
                        T_bf[cb][:, p * H:(p + 1) * H],
                        start=(cb == 0 and p in (0, 64)), stop=False)
        if g * PG + PG - 1 == 63:
            do_ac(0, 512)    # close bank A mid-stream, then exp it
            do_exp(0, 64)
    do_ac(512, Q * H)
    do_exp(64, Q)

    # ---------------- output matmuls + normalize ----------------
    # pot[q, j] = sum_k exp[k, h, q]^T v_aug[k, h, j]; col DH = denominator
    out_sb = const.tile([Q, D], FP32, tag="osb", name="osb")
    for h in range(H):
        pot = psum_out.tile([Q, DH + 1], FP32, tag="pot", name="pot")
        for kt in range(KT):
            nc.tensor.matmul(
                pot, exp_sb[kt][:, h, :], v_aug[kt][:, h, :],
                start=(kt == 0), stop=(kt == KT - 1))
        rec = const.tile([Q, 1], FP32, tag=f"rec{h}", name=f"rec{h}")
        nc.vector.reciprocal(out=rec, in_=pot[:, DH:DH + 1])
        nc.vector.tensor_scalar_mul(
            out=out_sb[:, h * DH:(h + 1) * DH], in0=pot[:, 0:DH], scalar1=rec)

    nc.sync.dma_start(out=out, in_=out_sb)
    ctx.close()


def build_program():
    nc = bacc.Bacc(
        "TRN2", target_bir_lowering=False, debug=False,
        num_devices=NCORES)
    ins = {
        "posT": nc.dram_tensor(
            "posT", [128, Q * CB * L], BF16, kind="ExternalInput").ap(),
        "wb1": nc.dram_tensor("wb1", [D, WBC1], BF16, kind="ExternalInput").ap(),
        "wb2": nc.dram_tensor("wb2", [D, WBC2], BF16, kind="ExternalInput").ap(),
        "cols": nc.dram_tensor(
            "cols", [128, 12], FP32, kind="ExternalInput").ap(),
        "bvb": nc.dram_tensor(
            "bvb", [128, D], FP32, kind="ExternalInput").ap(),
    }
    outs = {
        "out": nc.dram_tensor("out", [Q, D], FP32, kind="ExternalOutput").ap(),
    }
    with tile.TileContext(nc) as tc:
        build_kernel_body(tc, outs, ins)
    nc.compile()
    return nc


def shard_inputs(inputs):
    """Full inputs -> list of 8 per-core input dicts (numpy, contiguous).

    All transposes/casts/packing happen here so the device kernel streams
    every tensor in its natural consumption order.
    """
    import ml_dtypes
    bf = ml_dtypes.bfloat16
    f32 = lambda a: np.ascontiguousarray(np.asarray(a), dtype=np.float32)
    pos = np.asarray(inputs["pos"], dtype=np.float32)
    key, query, value = f32(inputs["key"]), f32(inputs["query"]), f32(inputs["value"])
    mask = f32(inputs["key_mask"])
    Wk, Wq, Wv, Wr = (f32(inputs[k]) for k in ("Wk", "Wq", "Wv", "Wr"))
    bk, bq, bv = f32(inputs["bk"]), f32(inputs["bq"]), f32(inputs["bv"])
    u, v = f32(inputs["u"]), f32(inputs["v"])

    cols = np.zeros((128, 12), np.float32)
    uu, vv = bq + u.reshape(-1), bq + v.reshape(-1)
    cols[:, 0], cols[:, 1] = bk[:128], bk[128:]
    cols[:, 2], cols[:, 3] = uu[:128], uu[128:]
    cols[:, 4], cols[:, 5] = vv[:128], vv[128:]
    bvb = np.ascontiguousarray(np.broadcast_to(bv, (128, D))).astype(np.float32)
    wb2_by_b = {}

    in_maps = []
    for c in range(NCORES):
        b, q0 = c // 4, (c % 4) * Q
        colsb = cols.copy()
        mb = (mask[b] - 1.0) * 1e15
        colsb[:, 6], colsb[:, 7], colsb[:, 8] = mb[:128], mb[128:256], mb[256:]
        wb1m = np.ascontiguousarray(np.concatenate(
            [Wq.T, Wr, query[b, q0:q0 + Q].T], axis=1).astype(bf))
        if b not in wb2_by_b:
            wb2_by_b[b] = np.ascontiguousarray(np.concatenate(
                [Wk.T, Wv.T, key[b].T, value[b].T], axis=1).astype(bf))
        pb = pos[b, q0:q0 + Q].astype(bf)
        pr = np.ascontiguousarray(
            pb.reshape(Q, L, CB, 128).transpose(3, 0, 2, 1)
        ).reshape(128, Q * CB * L)
        in_maps.append({
            "posT": pr, "wb1": wb1m, "wb2": wb2_by_b[b],
            "cols": colsb, "bvb": bvb,
        })
    return in_maps


_CACHED = {}


def kernel(**inputs):
    from concourse.bass_utils import run_bass_kernel_spmd

    if "nc" not in _CACHED:
        _CACHED["nc"] = build_program()
    nc = _CACHED["nc"]
    in_maps = shard_inputs(inputs)
    res = run_bass_kernel_spmd(nc, in_maps, core_ids=list(range(NCORES)))
    out = np.zeros((B, L, D), dtype=np.float32)
    for c in range(NCORES):
        b, q0 = c // 4, (c % 4) * Q
        out[b, q0:q0 + Q] = res.results[c]["out"]
    return out


# revision 7
# speedup vs baseline: 4.2176x; 1.0998x over previous
"""Trainium2 Bass kernel for relative-position multi-head attention.

Shapes (hardcoded): B=2, L=384, D=256, H=8, DH=32.
Sharding: 8 cores; core c handles batch b=c//4, query rows [(c%4)*96, +96).
Pure data-parallel SPMD - no collectives.

Math (per batch b, query q):
  q/k/v projections: x @ W.T + bias
  A_C[h,k] = (q_h+u_h) . k_h[k]
  B_D[h,k] = (q_h+v_h) . (Wr_h @ pos[q,k] + br_h)
           = (Wr_h^T (q_h+v_h)) . pos[q,k]   + const(h,q)   [br term is
             k-independent -> cancels in softmax -> dropped]
  score    = (A_C + B_D)/sqrt(DH) - (1-mask[k])*1e15
  out      = softmax_k(score) @ v

Key restructurings:
  * r = pos @ Wr.T (38 GFLOP) is never materialized; instead
    T[q] = Wr_h^T (q+v)  (a [256,8] matrix per query) and
    B_D = posT @ T  (1.2 GFLOP).
  * ALL layout work happens on the host in shard_inputs: pos arrives as
    bf16 [128(d%128), Q, 2(d//128), L] (partition-major, each pos DMA is a
    per-partition-contiguous stream), weights arrive bf16 pre-transposed.
    wb1 (q-proj) rides the sync ring AHEAD of the pos stream so the
    critical qv->T chain is fed first; everything else on the Act ring.
  * scores live in PSUM as [k-partitions, (pair,h)-free]. The B_D pair
    stream OPENS each psum bank (start=True on pair 0/64) and A_C -
    emitted after each bank's last pair - accumulates on top and CLOSES
    it, so B_D never waits on A_C/k-proj setup.
  * T computed per-head into an h*128-pitch psum then one strided
    Vector copy to (q,h) order; A_C uses a block-diagonal (q+u) operand
    built on GpSimd (dead time until the end of the stream).
  * softmax denominator via a ones-column appended to v_proj; exp on ACT
    with mask bias + 1/sqrt(dh) scale fused; the output matmul consumes
    exp as contiguous lhsT yielding out^T per head directly (zero
    transposes anywhere); output is split at the q=64 bank boundary so
    exp(bank B) on ACT overlaps bank-A output matmuls on PE.
"""

import sys

for _p in ("/opt/trn_rl_repo", "/root/.axon_site/_ro/trn_rl_repo"):
    if _p not in sys.path:
        sys.path.append(_p)

import numpy as np

import concourse.bass as bass
import concourse.mybir as mybir
import concourse.tile as tile
from concourse import bacc

FP32 = mybir.dt.float32
BF16 = mybir.dt.bfloat16

B, L, D, H = 2, 384, 256, 8
DH = D // H            # 32
Q = 96                 # queries per core
KT = L // 128          # 3 k-tiles
CB = D // 128          # 2 contraction blocks
NCORES = 8
SCALE = 1.0 / np.sqrt(DH)
PG = 8                 # pairs per pos DMA group
NG = Q // PG           # 12 groups

# host-packed bf16 weight tensors:
#   wb1 [D, 352]:  Wq.T | query.T     (q-proj critical path, sync ring)
W1_WQ, W1_QRY = 0, D
WBC1 = D + Q
#   wb2 [D, 1280]: Wk.T | Wv.T | key.T | value.T
W2_WK, W2_WV, W2_KEY, W2_VAL = 0, D, 2 * D, 2 * D + L
WBC2 = 2 * D + 2 * L
#   wrh [DH, H*D]: wrh[i, h*D+d] = Wr[h*DH+i, d]

# "cols" [128, 12] f32 per-partition columns:
#   0,1 = bk | 2,3 = bq+u | 4,5 = bq+v | 6,7,8 = (mask-1)*1e15 per k-tile


def build_kernel_body(tc, outs, ins):
    """Emit the per-core program. outs/ins are dicts of DRAM APs."""
    from contextlib import ExitStack
    ctx = ExitStack()
    pool = lambda **kw: ctx.enter_context(tc.tile_pool(**kw))
    nc = tc.nc
    posT = ins["posT"]      # [128, Q*CB*L] bf16  (p, q, cb, k)
    wb1 = ins["wb1"]        # [D, WBC1] bf16
    wb2 = ins["wb2"]        # [D, WBC2] bf16
    wrh = ins["wrh"]        # [DH, H*D] bf16
    cols = ins["cols"]      # [128, 12] f32
    bvb = ins["bvb"]        # [128, D] f32 (bv broadcast to 128 rows)
    out = outs["out"]       # [Q, D] f32

    const = pool(name="const", bufs=1)
    pos_pool = pool(name="pos", bufs=8)
    psum_big = pool(name="psum_big", bufs=3, space="PSUM")
    psum_out = pool(name="psum_out", bufs=2, space="PSUM")

    # ---------------- setup loads ----------------
    wb1_t = [const.tile([128, WBC1], BF16, tag=f"wb1_{cb}", name=f"wb1_{cb}")
             for cb in range(CB)]
    for cb in range(CB):
        nc.sync.dma_start(out=wb1_t[cb], in_=wb1[cb * 128:(cb + 1) * 128, :])
    cols_t = const.tile([128, 12], FP32, tag="cols", name="cols")
    nc.scalar.dma_start(out=cols_t, in_=cols)
    wrh_t = const.tile([DH, H * D], BF16, tag="wrh", name="wrh")
    nc.scalar.dma_start(out=wrh_t, in_=wrh)
    wb2_t = [const.tile([128, WBC2], BF16, tag=f"wb2_{cb}", name=f"wb2_{cb}")
             for cb in range(CB)]
    for cb in range(CB):
        nc.scalar.dma_start(out=wb2_t[cb], in_=wb2[cb * 128:(cb + 1) * 128, :])
    bvb_t = const.tile([128, D], FP32, tag="bvb", name="bvb")
    nc.scalar.dma_start(out=bvb_t, in_=bvb)

    # ---------------- q-projection -> qv_h (Vector) / QU (GpSimd) -------
    qp_sb = [const.tile([128, Q], FP32, tag=f"qp{dt}", name=f"qp{dt}")
             for dt in range(CB)]
    qv_h = [const.tile([DH, Q], BF16, tag=f"qvh{h}", name=f"qvh{h}")
            for h in range(H)]
    QU = [const.tile([128, Q * H], BF16, tag=f"QU{dt}", name=f"QU{dt}")
          for dt in range(CB)]
    qu_v = [QU[dt].rearrange("p (q h) -> p q h", h=H) for dt in range(CB)]
    for dt in range(CB):
        nc.gpsimd.memset(QU[dt], 0.0)
    for dt in range(CB):
        ps = psum_big.tile([128, 1024], FP32, tag="big", name="ps_qp")
        for cb in range(CB):
            nc.tensor.matmul(
                ps[:, :Q], wb1_t[cb][:, W1_WQ + dt * 128:W1_WQ + (dt + 1) * 128],
                wb1_t[cb][:, W1_QRY:W1_QRY + Q],
                start=(cb == 0), stop=(cb == CB - 1))
        nc.vector.tensor_copy(out=qp_sb[dt], in_=ps[:, :Q])
        for hh in range(4):
            h = dt * 4 + hh
            sl = slice(hh * DH, (hh + 1) * DH)
            nc.vector.tensor_scalar_add(
                out=qv_h[h], in0=qp_sb[dt][sl, :],
                scalar1=cols_t[sl, 4 + dt:5 + dt])
            nc.gpsimd.tensor_scalar_add(
                out=qu_v[dt][sl, :, h], in0=qp_sb[dt][sl, :],
                scalar1=cols_t[sl, 2 + dt:3 + dt])

    # ---------------- T matrices (B_D weights) ----------------
    # per-head matmuls into h*128-pitch psum, then one strided copy to
    # T_bf[cb][d(128), (q,h)] bf16
    T_bf = [const.tile([128, Q * H], BF16, tag=f"T{cb}", name=f"Tbf{cb}")
            for cb in range(CB)]
    for cbo in range(CB):
        ps = psum_big.tile([128, 1024], FP32, tag="big", name="ps_T")
        for h in range(H):
            nc.tensor.matmul(
                ps[:, h * 128:h * 128 + Q],
                wrh_t[:, h * D + cbo * 128:h * D + (cbo + 1) * 128],
                qv_h[h], start=True, stop=True)
        nc.vector.tensor_copy(
            out=T_bf[cbo].rearrange("p (q h) -> p h q", h=H),
            in_=ps.rearrange("p (h x) -> p h x", x=128)[:, :, 0:Q])

    # ---------------- k-projection (for the trailing A_C) ----------------
    kpT_b = [const.tile([128, L], BF16, tag=f"kpb{dt}", name=f"kpb{dt}")
             for dt in range(CB)]
    for dt in range(CB):
        ps = psum_big.tile([128, 1024], FP32, tag="big", name="ps_kp")
        for cb in range(CB):
            nc.tensor.matmul(
                ps[:, :L], wb2_t[cb][:, W2_WK + dt * 128:W2_WK + (dt + 1) * 128],
                wb2_t[cb][:, W2_KEY:W2_KEY + L],
                start=(cb == 0), stop=(cb == CB - 1))
        nc.vector.tensor_scalar_add(
            out=kpT_b[dt], in0=ps[:, :L], scalar1=cols_t[:, 0 + dt:1 + dt])

    # ---------------- v_proj + ones column -> v_aug ----------------
    v_aug = []
    for kt in range(KT):
        ps = psum_out.tile([128, D], FP32, tag="pot", name="ps_v")
        for cb in range(CB):
            nc.tensor.matmul(
                ps, wb2_t[cb][:, W2_VAL + kt * 128:W2_VAL + (kt + 1) * 128],
                wb2_t[cb][:, W2_WV:W2_WV + D],
                start=(cb == 0), stop=(cb == CB - 1))
        va = const.tile([128, H, DH + 1], BF16, tag=f"va{kt}", name=f"va{kt}")
        nc.vector.memset(va, 1.0)
        nc.vector.tensor_add(
            out=va[:, :, 0:DH], in0=ps.rearrange("p (h d) -> p h d", h=H),
            in1=bvb_t.rearrange("p (h d) -> p h d", h=H))
        v_aug.append(va)

    # ---------------- scores PSUM ----------------
    # per k-tile: [128, 1024] f32 (2 banks); col 8q+h holds (pair q, head h)
    scores = [psum_big.tile([128, 1024], FP32, tag="big", name=f"scores{kt}")
              for kt in range(KT)]

    def do_ac(c0, c1):
        for kt in range(KT):
            for cb in range(CB):
                nc.tensor.matmul(
                    scores[kt][:, c0:c1],
                    kpT_b[cb][:, kt * 128:(kt + 1) * 128],
                    QU[cb][:, c0:c1],
                    start=False, stop=(cb == CB - 1))

    # exp_sb h-major [128, (h,q)] so output matmuls get contiguous lhsT
    exp_sb = [const.tile([128, H, Q], BF16, tag=f"exp{kt}", name=f"exp{kt}")
              for kt in range(KT)]
    ex_qh = [exp_sb[kt].rearrange("p h q -> p q h") for kt in range(KT)]
    sc_qh = [scores[kt][:, :Q * H].rearrange("p (q h) -> p q h", h=H)
             for kt in range(KT)]

    def do_exp(r0, r1):
        for kt in range(KT):
            nc.scalar.activation(
                out=ex_qh[kt][:, r0:r1, :], in_=sc_qh[kt][:, r0:r1, :],
                func=mybir.ActivationFunctionType.Exp,
                bias=cols_t[:, 6 + kt:7 + kt], scale=float(SCALE))

    out_sb = const.tile([Q, D], FP32, tag="osb", name="osb")

    def do_out(r0, r1):
        # pot[q-r0, j] = sum_k exp[k, h, q] v_aug[k, h, j]; col DH = denom
        n = r1 - r0
        for h in range(H):
            pot = psum_out.tile([n, DH + 1], FP32, tag="pot", name="pot")
            for kt in range(KT):
                nc.tensor.matmul(
                    pot, exp_sb[kt][:, h, r0:r1], v_aug[kt][:, h, :],
                    start=(kt == 0), stop=(kt == KT - 1))
            rec = const.tile([n, 1], FP32, tag=f"rec{h}_{r0}", name="rec")
            nc.vector.reciprocal(out=rec, in_=pot[:, DH:DH + 1])
            nc.vector.tensor_scalar_mul(
                out=out_sb[r0:r1, h * DH:(h + 1) * DH],
                in0=pot[:, 0:DH], scalar1=rec)

    # ---------------- pos stream + B_D matmuls ----------------
    GSZ = PG * CB * L   # bf16 elems per partition per group
    for g in range(NG):
        pg = pos_pool.tile([128, GSZ], BF16, tag="posg", name=f"pos{g}")
        nc.sync.dma_start(out=pg, in_=posT[:, g * GSZ:(g + 1) * GSZ])
        for i in range(PG):
            p = g * PG + i
            for cb in range(CB):
                base = i * CB * L + cb * L
                for kt in range(KT):
                    nc.tensor.matmul(
                        scores[kt][:, p * H:(p + 1) * H],
                        pg[:, base + kt * 128: base + kt * 128 + 128],
                        T_bf[cb][:, p * H:(p + 1) * H],
                        start=(cb == 0 and p in (0, 64)), stop=False)
        if g * PG + PG - 1 == 63:
            do_ac(0, 512)    # close bank A mid-stream, exp it on ACT
            do_exp(0, 64)
    do_ac(512, Q * H)        # close bank B
    do_exp(64, Q)            # on ACT, overlaps bank-A output matmuls below
    do_out(0, 64)
    do_out(64, Q)

    nc.sync.dma_start(out=out, in_=out_sb)
    ctx.close()


def build_program():
    nc = bacc.Bacc(
        "TRN2", target_bir_lowering=False, debug=False,
        num_devices=NCORES)
    ins = {
        "posT": nc.dram_tensor(
            "posT", [128, Q * CB * L], BF16, kind="ExternalInput").ap(),
        "wb1": nc.dram_tensor("wb1", [D, WBC1], BF16, kind="ExternalInput").ap(),
        "wb2": nc.dram_tensor("wb2", [D, WBC2], BF16, kind="ExternalInput").ap(),
        "wrh": nc.dram_tensor(
            "wrh", [DH, H * D], BF16, kind="ExternalInput").ap(),
        "cols": nc.dram_tensor(
            "cols", [128, 12], FP32, kind="ExternalInput").ap(),
        "bvb": nc.dram_tensor(
            "bvb", [128, D], FP32, kind="ExternalInput").ap(),
    }
    outs = {
        "out": nc.dram_tensor("out", [Q, D], FP32, kind="ExternalOutput").ap(),
    }
    with tile.TileContext(nc) as tc:
        build_kernel_body(tc, outs, ins)
    nc.compile()
    return nc


def shard_inputs(inputs):
    """Full inputs -> list of 8 per-core input dicts (numpy, contiguous).

    All transposes/casts/packing happen here so the device kernel streams
    every tensor in its natural consumption order.
    """
    import ml_dtypes
    bf = ml_dtypes.bfloat16
    f32 = lambda a: np.ascontiguousarray(np.asarray(a), dtype=np.float32)
    pos = np.asarray(inputs["pos"], dtype=np.float32)
    key, query, value = f32(inputs["key"]), f32(inputs["query"]), f32(inputs["value"])
    mask = f32(inputs["key_mask"])
    Wk, Wq, Wv, Wr = (f32(inputs[k]) for k in ("Wk", "Wq", "Wv", "Wr"))
    bk, bq, bv = f32(inputs["bk"]), f32(inputs["bq"]), f32(inputs["bv"])
    u, v = f32(inputs["u"]), f32(inputs["v"])

    cols = np.zeros((128, 12), np.float32)
    uu, vv = bq + u.reshape(-1), bq + v.reshape(-1)
    cols[:, 0], cols[:, 1] = bk[:128], bk[128:]
    cols[:, 2], cols[:, 3] = uu[:128], uu[128:]
    cols[:, 4], cols[:, 5] = vv[:128], vv[128:]
    bvb = np.ascontiguousarray(np.broadcast_to(bv, (128, D))).astype(np.float32)
    wrh = np.ascontiguousarray(
        Wr.reshape(H, DH, D).transpose(1, 0, 2).reshape(DH, H * D).astype(bf))
    wb2_by_b = {}

    in_maps = []
    for c in range(NCORES):
        b, q0 = c // 4, (c % 4) * Q
        colsb = cols.copy()
        mb = (mask[b] - 1.0) * 1e15
        colsb[:, 6], colsb[:, 7], colsb[:, 8] = mb[:128], mb[128:256], mb[256:]
        wb1m = np.ascontiguousarray(np.concatenate(
            [Wq.T, query[b, q0:q0 + Q].T], axis=1).astype(bf))
        if b not in wb2_by_b:
            wb2_by_b[b] = np.ascontiguousarray(np.concatenate(
                [Wk.T, Wv.T, key[b].T, value[b].T], axis=1).astype(bf))
        pb = pos[b, q0:q0 + Q].astype(bf)
        pr = np.ascontiguousarray(
            pb.reshape(Q, L, CB, 128).transpose(3, 0, 2, 1)
        ).reshape(128, Q * CB * L)
        in_maps.append({
            "posT": pr, "wb1": wb1m, "wb2": wb2_by_b[b], "wrh": wrh,
            "cols": colsb, "bvb": bvb,
        })
    return in_maps


_CACHED = {}


def kernel(**inputs):
    from concourse.bass_utils import run_bass_kernel_spmd

    if "nc" not in _CACHED:
        _CACHED["nc"] = build_program()
    nc = _CACHED["nc"]
    in_maps = shard_inputs(inputs)
    res = run_bass_kernel_spmd(nc, in_maps, core_ids=list(range(NCORES)))
    out = np.zeros((B, L, D), dtype=np.float32)
    for c in range(NCORES):
        b, q0 = c // 4, (c % 4) * Q
        out[b, q0:q0 + Q] = res.results[c]["out"]
    return out
